# revision 1
# baseline (speedup 1.0000x reference)
"""
Trainium2 Bass kernel for: MultiStepLIF (T=4) -> depthwise 3x3 conv -> BatchNorm2d
(training-mode batch stats), data-parallel over batch across 8 NeuronCores.

Contract: kernel(**inputs) takes FULL numpy inputs
    x: [4, 16, 384, 32, 32] f32, w: [384, 1, 3, 3] f32, gamma/beta: [384] f32
and returns the FULL output [4, 16, 384, 32, 32] f32.

Dispatch-dominated problem: the per-launch cost through the PJRT layer is
driven almost entirely by I/O buffer bytes (a pure-copy kernel with the same
I/O signature times identically to the full fused kernel). Hence v3:
  - one strided DMA per channel chunk for input (3 loads) and output
    (3 stores) instead of per-(t,b,chunk) tiles;
  - f16 DRAM output (halves output bytes; host upcasts to f32; adds ~6e-4
    rel rounding against the 2e-2 gate);
  - BN applied in-place on the f16 staging buffer (no output staging copy).

Per core (batch shard of 2):
  - LIF scan in doubled-membrane form u_t = u_{t-1}*0.5 + x_t (one DVE
    scalar_tensor_tensor per step; *0.5 is exact so u/2 reproduces the
    reference fp32 membrane bit-for-bit on this data — validated), spike
    s = (u >= 2) as fp16 {0,1}; hard reset in one STT op u <- (u<2)*u.
  - Spikes copied (ScalarE) into a zero-bordered flat layout [1 + 34*33]:
    33-wide rows (col 32 always zero) + zero rows top/bottom. Any 3x3-shifted
    window is then a CONTIGUOUS 1-D slice, with the zero column providing
    exact zero-padding at row wrap.
  - Depthwise conv: per 32-row tile, one 4-bank PSUM tile, 4x 8-row segments;
    9 accumulating TensorE matmuls per segment with fp8e4 DoubleRow pair-split
    diagonal weights (w*64 = a_fp8 + b_fp8, K virtually 256 via a step-0
    broadcast middle dim on the rhs) => ~2x PE throughput, ~1.1e-3 weight err.
  - PSUM evacuation: ONE ScalarE Copy per tile (4-D AP skips the junk column,
    scale=1/64) with fused accum_out => BN sums; sum-of-squares on DVE fp16
    scalar_tensor_tensor with accum_out.
  - Sync-BN: AllReduce-add of [128, 6] per-channel partial sums (required:
    per-shard BN stats measure 7.1e-2 rel err vs the 2e-2 gate).
  - BN apply y*a + b split DVE/ScalarE in-place on the f16 staging buffer,
    then one strided DMA per channel chunk stores [128, T, BS, HW] f16.

Built on bacc.Bacc + TileContext; nc.compile() splits multi-semaphore waits
(TRN2 allows 1 wait per data instruction — raw bass.Bass fails in walrus).
"""

import numpy as np

# ---- problem constants (hardcoded; kernel.py must be self-contained) ----
T = 4
B = 16
C = 384
H = 32
W = 32
HW = H * W
NCORES = 8
BS = B // NCORES          # batch per core = 2
NCHUNK = C // 128         # 3 channel chunks of 128
NTILE = NCHUNK * BS * T   # 24 [128, 1024] output tiles per core
NTOT = T * B * HW         # 65536 samples per channel for BN stats
BN_EPS = 1e-5
WSCALE = 64.0             # pow2 weight prescale for fp8 pair-split
NSEG = 4                  # conv row-segments per 32-row tile (8 rows each)
SROWS = H // NSEG         # 8
ROWB = W + 1              # 33: padded row pitch (one zero col)
SEGN = SROWS * ROWB - 1   # 263 matmul output columns per segment

_CACHE = {}


def build_program(n_cores=NCORES, with_collective=True, use_dr=True, taps=9,
                  repeat=1):
    """Build the Bass/Tile program (SPMD, one NeuronCore's instruction stream).

    v2: batched I/O — one input DMA and one output DMA per channel chunk
    ([128, BS, T, HW] with strided DRAM APs) instead of per-(t,b,ch) tiles;
    f32 conv staging with in-place BN apply (no separate output staging).
    """
    import concourse.bass as bass
    import concourse.bacc as bacc
    import concourse.tile as tile
    import concourse.mybir as mybir

    f32 = mybir.dt.float32
    f16 = mybir.dt.float16
    f8 = mybir.dt.float8e4
    wdt = f8 if use_dr else f16
    AL = mybir.AluOpType
    AF = mybir.ActivationFunctionType
    AX = mybir.AxisListType

    nc = bacc.Bacc("TRN2", target_bir_lowering=False, debug=False,
                   num_devices=n_cores)

    x_d = nc.dram_tensor("x", [T, BS, C, HW], f32, kind="ExternalInput").ap()
    wshape = [128, NCHUNK * 9, 2, 128] if use_dr else [128, NCHUNK * 9, 128]
    wd_d = nc.dram_tensor("wd", wshape, wdt, kind="ExternalInput").ap()
    gb_d = nc.dram_tensor("gb", [128, 6], f32, kind="ExternalInput").ap()
    # f16 output: halves per-dispatch output bytes (the dominant dispatch
    # cost through the PJRT/axon layer); host upcasts to f32 after gather.
    # Adds ~6e-4 rel rounding vs the 2e-2 gate.
    y_d = nc.dram_tensor("y", [T, BS, C, HW], f16, kind="ExternalOutput").ap()

    SPLEN = 1 + 34 * ROWB  # leading zero elem + 34 padded rows

    def tidx(ch, b, t):
        # t-major within a chunk so the [t, b] DRAM dims merge to one
        # stride-C*HW dim (DMA AP balancer allows at most 3 dims per side)
        return (ch * T + t) * BS + b

    def chunk_ap(dram, ch):
        """[128(part=C slice), T, BS, HW] strided view of [T,BS,C,HW] dram."""
        return bass.AP(
            tensor=dram.tensor,
            offset=dram.offset + ch * 128 * HW,
            ap=[[HW, 128], [BS * C * HW, T], [C * HW, BS], [1, HW]])

    with tile.TileContext(nc) as tc:
        with (
            tc.tile_pool(name="const", bufs=1) as cpool,
            tc.tile_pool(name="xin", bufs=2) as xpool,
            tc.tile_pool(name="lif", bufs=3) as lpool,
            tc.tile_pool(name="outp", bufs=4) as opool,
            tc.tile_pool(name="ps", bufs=2, space="PSUM") as pspool,
            tc.tile_pool(name="dram", bufs=1, space="DRAM") as dpool,
        ):
            # ---- constants ----
            wsb = cpool.tile(wshape, wdt, name="wsb")
            nc.sync.dma_start(out=wsb[:], in_=wd_d[:])
            gbsb = cpool.tile([128, 6], f32, name="gbsb")
            nc.sync.dma_start(out=gbsb[:], in_=gb_d[:])
            # persistent padded spike buffers (borders zeroed once)
            NSP = 6
            sp_bufs = []
            for i in range(NSP):
                spb = cpool.tile([128, SPLEN], wdt, name=f"spb{i}")
                nc.vector.memset(spb[:], 0.0)
                sp_bufs.append(spb)

            # f16 conv output staging (BN applied in-place, DMA'd out direct)
            y_all = cpool.tile([128, NTILE, HW], f16, name="y_all")
            ssum = cpool.tile([128, NTILE], f32, name="ssum")
            ssq = cpool.tile([128, NTILE], f32, name="ssq")
            loc = cpool.tile([128, 6], f32, name="loc")
            gsum = cpool.tile([128, 6], f32, name="gsum")
            prm = cpool.tile([128, 48], f32, name="prm")

            # ---- phase 1: LIF + depthwise conv + local BN partial sums ----
            # (repeat>1 unrolls the whole body; timing-only diagnostic to
            # measure the kernel's marginal device time per execution)
            it = 0
            for rep in range(repeat):
              for ch in range(NCHUNK):
                xsb = xpool.tile([128, T, BS, HW], f32, tag="xsb",
                                 name="xsb")
                nc.sync.dma_start(out=xsb[:], in_=chunk_ap(x_d, ch))
                for b in range(BS):
                    u_prev = None
                    for t in range(T):
                        xt = xsb[:, t, b, :]

                        if t == 0:
                            u = xt  # u_0 = x_0 (threshold doubles to 2.0)
                        else:
                            u = lpool.tile([128, HW], f32, tag="u", name="u")
                            nc.vector.scalar_tensor_tensor(
                                out=u[:], in0=u_prev[:], scalar=0.5,
                                in1=xt[:], op0=AL.mult, op1=AL.add)

                        s2 = lpool.tile([128, HW], f16, tag="s2", name="s2")
                        nc.vector.tensor_scalar(s2[:], u[:], 2.0, None,
                                                AL.is_ge)

                        sp = sp_bufs[it % NSP]
                        it += 1
                        # interior of padded grid: rows 1..32, cols 0..31
                        # (ScalarE copy; GPSIMD data ops hit a walrus limit
                        # on sync-wait count)
                        grid = sp[:, 1:].rearrange("p (a b) -> p a b", a=34)
                        nc.scalar.activation(
                            out=grid[:, 1:33, 0:W],
                            in_=s2[:].rearrange("p (h w) -> p h w", h=H),
                            func=AF.Copy)

                        if t < T - 1:
                            # hard reset in one op: u <- (u < 2) * u
                            nc.vector.scalar_tensor_tensor(
                                out=u[:], in0=u[:], scalar=2.0, in1=u[:],
                                op0=AL.is_lt, op1=AL.mult)
                            u_prev = u

                        # conv: 4x8-row segments x 9 taps, fp8 DoubleRow,
                        # one 4-bank PSUM tile per 32-row output tile
                        ti = tidx(ch, b, t)
                        ps = pspool.tile([128, NSEG, 512], f32, tag="ps",
                                         name="ps")
                        for si in range(NSEG):
                            r0 = si * SROWS
                            for k in range(taps):
                                dh, dw = k // 3, k % 3
                                # flat offset of input window in sp
                                off = 1 + (r0 + dh) * ROWB + dw - 1
                                base = sp[:, off:off + SEGN]
                                if use_dr:
                                    rhs = bass.AP(
                                        tensor=base.tensor,
                                        offset=base.offset,
                                        ap=[base.ap[0], [0, 2], base.ap[1]])
                                    pm = mybir.MatmulPerfMode.DoubleRow
                                    lhsT = wsb[:, ch * 9 + k, :, :]
                                else:
                                    rhs = base
                                    pm = None
                                    lhsT = wsb[:, ch * 9 + k, :]
                                nc.tensor.matmul(
                                    out=ps[:, si, 0:SEGN], lhsT=lhsT, rhs=rhs,
                                    start=(k == 0), stop=(k == taps - 1),
                                    perf_mode=pm)
                        # evacuate whole tile (undo WSCALE) + fused sum:
                        # view each segment's 263 cols as [8, 33], keep 32
                        ps4 = bass.AP(
                            tensor=ps.tensor, offset=ps.offset,
                            ap=[ps.ap[0], [512, NSEG], [ROWB, SROWS], [1, W]])
                        y4 = y_all[:, ti, :].rearrange(
                            "p (s r w) -> p s r w", s=NSEG, r=SROWS)
                        nc.scalar.activation(
                            out=y4, in_=ps4, func=AF.Copy,
                            scale=(1.0 / WSCALE) if use_dr else 1.0,
                            accum_out=ssum[:, ti:ti + 1])
                        # sum-of-squares per full tile on DVE (fp16 2x)
                        sc = opool.tile([128, HW], f16, tag="sc", name="sc")
                        nc.vector.scalar_tensor_tensor(
                            out=sc[:], in0=y_all[:, ti, :], scalar=1.0,
                            in1=y_all[:, ti, :], op0=AL.mult, op1=AL.mult,
                            accum_out=ssq[:, ti:ti + 1])

            # ---- phase 2: reduce partials, sync-BN all-reduce ----
            NQ = BS * T              # 8 cols per chunk
            for ch in range(NCHUNK):
                nc.vector.tensor_reduce(out=loc[:, ch:ch + 1],
                                        in_=ssum[:, ch * NQ:(ch + 1) * NQ],
                                        axis=AX.X, op=AL.add)
                nc.vector.tensor_reduce(out=loc[:, 3 + ch:4 + ch],
                                        in_=ssq[:, ch * NQ:(ch + 1) * NQ],
                                        axis=AX.X, op=AL.add)

            if with_collective:
                cin = dpool.tile([128, 6], f32, name="cin")
                cout = dpool.tile([128, 6], f32, name="cout")
                nc.gpsimd.dma_start(out=cin[:], in_=loc[:])
                nc.gpsimd.collective_compute(
                    "AllReduce", AL.add,
                    replica_groups=[list(range(n_cores))],
                    ins=[cin.opt()], outs=[cout.opt()])
                nc.gpsimd.dma_start(out=gsum[:], in_=cout[:])
            else:
                nc.vector.tensor_copy(out=gsum[:], in_=loc[:])

            # ---- phase 3: BN parameters (tiny [128,3] ops) ----
            mu, m2 = prm[:, 0:3], prm[:, 3:6]
            sq, var = prm[:, 6:9], prm[:, 9:12]
            veps, sv = prm[:, 12:15], prm[:, 15:18]
            r0_, r0s = prm[:, 18:21], prm[:, 21:24]
            xr, tcr = prm[:, 24:27], prm[:, 27:30]
            r1_, aa = prm[:, 30:33], prm[:, 33:36]
            t2, bb = prm[:, 36:39], prm[:, 39:42]
            inv_n = 1.0 / float(NTOT)
            nc.vector.tensor_scalar(mu, gsum[:, 0:3], inv_n, None, AL.mult)
            nc.vector.tensor_scalar(m2, gsum[:, 3:6], inv_n, None, AL.mult)
            nc.vector.tensor_tensor(out=sq, in0=mu, in1=mu, op=AL.mult)
            nc.vector.tensor_tensor(out=var, in0=m2, in1=sq, op=AL.subtract)
            nc.vector.tensor_scalar(veps, var, BN_EPS, None, AL.add)
            nc.scalar.activation(out=sv, in_=veps, func=AF.Sqrt)
            nc.vector.reciprocal(out=r0_, in_=sv)
            # one Newton step: r1 = r0*(1.5 - 0.5*x*r0^2)
            nc.vector.tensor_tensor(out=r0s, in0=r0_, in1=r0_, op=AL.mult)
            nc.vector.tensor_tensor(out=xr, in0=veps, in1=r0s, op=AL.mult)
            nc.vector.tensor_scalar(tcr, xr, -0.5, 1.5, AL.mult, AL.add)
            nc.vector.tensor_tensor(out=r1_, in0=r0_, in1=tcr, op=AL.mult)
            # a = r1*gamma ; b = beta - mu*a
            nc.vector.tensor_tensor(out=aa, in0=r1_, in1=gbsb[:, 0:3],
                                    op=AL.mult)
            nc.vector.tensor_tensor(out=t2, in0=mu, in1=aa, op=AL.mult)
            nc.vector.tensor_tensor(out=bb, in0=gbsb[:, 3:6], in1=t2,
                                    op=AL.subtract)

            # ---- phase 4: apply BN in-place (split DVE / ScalarE), then one
            # strided DMA per channel chunk writes [128, BS, T, HW] at once ----
            n = 0
            for ch in range(NCHUNK):
                for ti in range(ch * BS * T, (ch + 1) * BS * T):
                    if n % 2 == 0:  # half DVE, half ScalarE
                        nc.vector.tensor_scalar(
                            y_all[:, ti, :], y_all[:, ti, :],
                            prm[:, 33 + ch:34 + ch],
                            prm[:, 39 + ch:40 + ch],
                            AL.mult, AL.add)
                    else:
                        nc.scalar.activation(
                            out=y_all[:, ti, :], in_=y_all[:, ti, :],
                            func=AF.Identity,
                            bias=prm[:, 39 + ch:40 + ch],
                            scale=prm[:, 33 + ch:34 + ch])
                    n += 1
                nc.sync.dma_start(
                    out=chunk_ap(y_d, ch),
                    in_=y_all[:, ch * BS * T:(ch + 1) * BS * T, :])
    nc.compile()
    return nc


def _host_prep(x, w, gamma, beta, use_dr=True):
    """Shard/transform the full inputs into per-core in_maps."""
    import ml_dtypes

    x = np.asarray(x, dtype=np.float32).reshape(T, B, C, HW)
    w = np.asarray(w, dtype=np.float32)
    gamma = np.asarray(gamma, dtype=np.float32)
    beta = np.asarray(beta, dtype=np.float32)

    w9 = w.reshape(C, 9)
    idx = np.arange(128)
    if use_dr:
        # pair-split: w*WSCALE = a_fp8 + b_fp8 (diagonal stationaries)
        f8 = ml_dtypes.float8_e4m3
        ws = (w9 * np.float32(WSCALE)).astype(np.float32)
        a = ws.astype(f8)
        bres = (ws - a.astype(np.float32)).astype(f8)
        wd = np.zeros((128, NCHUNK * 9, 2, 128), dtype=f8)
        for ch in range(NCHUNK):
            for k in range(9):
                wd[idx, ch * 9 + k, 0, idx] = a[ch * 128:(ch + 1) * 128, k]
                wd[idx, ch * 9 + k, 1, idx] = bres[ch * 128:(ch + 1) * 128, k]
    else:
        w16 = w9.astype(np.float16)
        wd = np.zeros((128, NCHUNK * 9, 128), dtype=np.float16)
        for ch in range(NCHUNK):
            for k in range(9):
                wd[idx, ch * 9 + k, idx] = w16[ch * 128:(ch + 1) * 128, k]

    gb = np.zeros((128, 6), dtype=np.float32)
    gb[:, 0:3] = gamma.reshape(NCHUNK, 128).T
    gb[:, 3:6] = beta.reshape(NCHUNK, 128).T

    in_maps = []
    for i in range(NCORES):
        xi = np.ascontiguousarray(x[:, i * BS:(i + 1) * BS])
        in_maps.append({"x": xi, "wd": wd, "gb": gb})
    return in_maps


def kernel(x, w, gamma, beta):
    from concourse.bass_utils import run_bass_kernel_spmd

    if "nc" not in _CACHE:
        _CACHE["nc"] = build_program()
    nc = _CACHE["nc"]

    in_maps = _host_prep(x, w, gamma, beta)
    res = run_bass_kernel_spmd(nc, in_maps, core_ids=list(range(NCORES)))

    out = np.empty((T, B, C, HW), dtype=np.float32)
    for i in range(NCORES):
        out[:, i * BS:(i + 1) * BS] = res.results[i]["y"]  # f16 -> f32 upcast
    return out.reshape(T, B, C, H, W)



# revision 2
# speedup vs baseline: 1719.9122x; 1719.9122x over previous
"""
Trainium2 Bass kernel: MultiStepLIF (T=4) -> depthwise 3x3 conv -> BatchNorm2d
(training-mode batch stats), data-parallel over batch across 8 NeuronCores.

Contract: kernel(**inputs) takes FULL numpy inputs
    x: [4, 16, 384, 32, 32] f32, w: [384, 1, 3, 3] f32, gamma/beta: [384] f32
and returns the FULL output [4, 16, 384, 32, 32] f32.

Per core (batch shard of 2):
  - LIF scan in doubled-membrane form u_t = u_{t-1}*0.5 + x_t (one DVE
    scalar_tensor_tensor per step; *0.5 is exact so u/2 reproduces the
    reference fp32 membrane bit-for-bit); hard reset u <- (u<2)*u in one STT.
  - The spike threshold (u >= 2, DVE is_ge) writes DIRECTLY into a
    zero-bordered padded grid [1 + 34*33] f16 via a strided 2-D AP: 33-wide
    rows (col 32 always zero) + zero rows top/bottom. Any 3x3-shifted window
    is then a CONTIGUOUS 1-D slice with exact zero padding at row wraps.
  - Depthwise conv: f16 diagonal weights, plain matmuls (fp8 DoubleRow
    pair-split measured ~2x SLOWER on this hardware and less accurate).
    Per 32-row tile: one 4-bank PSUM tile, 4x 8-row segments, k-outer loop
    (one stationary load per tap; 4 segment matmuls per load, PSUM
    accumulation interleaved across banks).
  - PSUM evacuation: one ScalarE Copy per tile with fused accum_out -> BN
    sums; sum of squares via ScalarE Square activation with accum_out.
  - Sync-BN: AllReduce-add of [128, 6] per-channel partial sums (required:
    per-shard BN stats measure 7.1e-2 rel err vs the 2e-2 gate).
  - BN parameters on-device (rsqrt + one Newton step); BN apply y*a + b
    split across DVE and Pool (gpsimd) engines in place on the f16 staging
    buffer, then one strided DMA per channel chunk stores the f16 output
    (host upcasts to f32; ~6e-4 rel rounding against the 2e-2 gate).

Engine balance (measured, marginal per-execution on HW ~44 us):
  DVE: LIF update/reset, spike thresholds, half of BN apply.
  ScalarE: PSUM evacuation + sum-of-squares (Square) with fused accums.
  Pool: other half of BN apply.  PE: 864 f16 matmuls (~6+ cols/cycle).
  DMA: 12.6 MB in + 6.3 MB out at ~1.36 TB/s/core (measured) ~ 14 us.

`repeat` unrolls the ENTIRE body (input DMA ... collective ... output DMA)
N times with double-buffered staging so (t_R - t_1)/(R-1) on interleaved
dispatches measures one full steady-state on-device execution with the
axon-tunnel RTT (~75-90 ms per blocking dispatch regardless of content)
cancelled out.
"""

import numpy as np

# ---- problem constants (hardcoded; kernel must be self-contained) ----
T = 4
B = 16
C = 384
H = 32
W = 32
HW = H * W
NCORES = 8
BS = B // NCORES          # batch per core = 2
NCHUNK = C // 128         # 3 channel chunks of 128
NTILE = NCHUNK * BS * T   # 24 [128, 1024] output tiles per core
NTOT = T * B * HW         # 65536 samples per channel for BN stats
BN_EPS = 1e-5
NSEG = 4                  # conv row-segments per 32-row tile (8 rows each)
SROWS = H // NSEG         # 8
ROWB = W + 1              # 33: padded row pitch (one zero col)
SEGN = SROWS * ROWB - 1   # 263 matmul output columns per segment
SPLEN = 1 + 34 * ROWB     # leading zero elem + 34 padded rows

_CACHE = {}


def build_program(n_cores=NCORES, with_collective=True, taps=9, repeat=1,
                  ssq_eng="s", reset_eng="v", evac_eng="s", bn_cycle="vp",
                  spike_eng="v", nsp=6, morder="k"):
    import concourse.bass as bass
    import concourse.bacc as bacc
    import concourse.tile as tile
    import concourse.mybir as mybir

    f32 = mybir.dt.float32
    f16 = mybir.dt.float16
    AL = mybir.AluOpType
    AF = mybir.ActivationFunctionType
    AX = mybir.AxisListType

    nc = bacc.Bacc("TRN2", target_bir_lowering=False, debug=False,
                   num_devices=n_cores)

    x_d = nc.dram_tensor("x", [T, BS, C, HW], f32, kind="ExternalInput").ap()
    wd_d = nc.dram_tensor("wd", [128, NCHUNK * 9, 128], f16,
                          kind="ExternalInput").ap()
    gb_d = nc.dram_tensor("gb", [128, 6], f32, kind="ExternalInput").ap()
    y_d = nc.dram_tensor("y", [T, BS, C, HW], f16, kind="ExternalOutput").ap()

    def eng(c):
        return {"v": nc.vector, "s": nc.scalar, "p": nc.gpsimd}[c]

    def tidx(ch, b, t):
        # t-major within a chunk so the [t, b] DRAM dims merge to one
        # stride-C*HW dim (DMA AP balancer allows at most 3 dims per side)
        return (ch * T + t) * BS + b

    def chunk_ap(dram, ch):
        """[128(part=C slice), T, BS, HW] strided view of [T,BS,C,HW] dram."""
        return bass.AP(
            tensor=dram.tensor,
            offset=dram.offset + ch * 128 * HW,
            ap=[[HW, 128], [BS * C * HW, T], [C * HW, BS], [1, HW]])

    with tile.TileContext(nc) as tc:
        with (
            tc.tile_pool(name="const", bufs=1) as cpool,
            tc.tile_pool(name="w", bufs=2) as wpool,
            tc.tile_pool(name="stage", bufs=2) as spool,
            tc.tile_pool(name="xin", bufs=2) as xpool,
            tc.tile_pool(name="lif", bufs=3) as lpool,
            tc.tile_pool(name="outp", bufs=4) as opool,
            tc.tile_pool(name="ps", bufs=2, space="PSUM") as pspool,
            tc.tile_pool(name="dram", bufs=1, space="DRAM") as dpool,
        ):
            # persistent padded spike grids (borders zeroed once; interior is
            # fully overwritten each use, borders are zero-invariant)
            sp_bufs = []
            for i in range(nsp):
                spb = cpool.tile([128, SPLEN], f16, name=f"spb{i}")
                nc.vector.memset(spb[:], 0.0)
                sp_bufs.append(spb)

            it = 0
            for rep in range(repeat):
                # ---- weights + gamma/beta ----
                wsb = wpool.tile([128, NCHUNK * 9, 128], f16, tag="wsb",
                                 name="wsb")
                nc.sync.dma_start(out=wsb[:], in_=wd_d[:])
                gbsb = wpool.tile([128, 6], f32, tag="gbsb", name="gbsb")
                nc.sync.dma_start(out=gbsb[:], in_=gb_d[:])
                # conv output staging + stats, double-buffered across reps so
                # rep r+1's phase 1 overlaps rep r's BN apply + output DMA
                y_all = spool.tile([128, NTILE, HW], f16, tag="y_all",
                                   name="y_all")
                ssum = spool.tile([128, NTILE], f32, tag="ssum", name="ssum")
                ssq = spool.tile([128, NTILE], f32, tag="ssq", name="ssq")
                loc = spool.tile([128, 6], f32, tag="loc", name="loc")
                gsum = spool.tile([128, 6], f32, tag="gsum", name="gsum")
                prm = spool.tile([128, 48], f32, tag="prm", name="prm")

                # ---- phase 1: LIF + depthwise conv + BN partial sums ----
                for ch in range(NCHUNK):
                    xsb = xpool.tile([128, T, BS, HW], f32, tag="xsb",
                                     name="xsb")
                    nc.sync.dma_start(out=xsb[:], in_=chunk_ap(x_d, ch))
                    for b in range(BS):
                        u_prev = None
                        for t in range(T):
                            xt = xsb[:, t, b, :]
                            if t == 0:
                                u = xt  # u_0 = x_0 (threshold doubles to 2.0)
                            else:
                                u = lpool.tile([128, HW], f32, tag="u",
                                               name="u")
                                nc.vector.scalar_tensor_tensor(
                                    out=u[:], in0=u_prev[:], scalar=0.5,
                                    in1=xt[:], op0=AL.mult, op1=AL.add)

                            # spike straight into padded grid rows 1..32,
                            # cols 0..31 (row pitch 33; col 32 stays zero)
                            sp = sp_bufs[it % nsp]
                            it += 1
                            grid = bass.AP(
                                tensor=sp.tensor, offset=sp.offset + 1 + ROWB,
                                ap=[sp.ap[0], [ROWB, H], [1, W]])
                            eng(spike_eng).tensor_scalar(grid, u[:], 2.0,
                                                         None, AL.is_ge)

                            if t < T - 1:
                                # hard reset in one op: u <- (u < 2) * u
                                un = lpool.tile([128, HW], f32, tag="u",
                                                name="u")
                                eng(reset_eng).scalar_tensor_tensor(
                                    out=un[:], in0=u[:], scalar=2.0, in1=u[:],
                                    op0=AL.is_lt, op1=AL.mult)
                                u_prev = un

                            # conv: k-outer (one weight load per tap), 4
                            # segments accumulate in one 4-bank PSUM tile
                            ti = tidx(ch, b, t)
                            ps = pspool.tile([128, NSEG, 512], f32, tag="ps",
                                             name="ps")
                            if morder == "k":
                                order = [(k, si) for k in range(taps)
                                         for si in range(NSEG)]
                            else:
                                order = [(k, si) for si in range(NSEG)
                                         for k in range(taps)]
                            for k, si in order:
                                dh, dw = k // 3, k % 3
                                lhsT = wsb[:, ch * 9 + k, :]
                                off = 1 + (si * SROWS + dh) * ROWB + dw - 1
                                rhs = sp[:, off:off + SEGN]
                                nc.tensor.matmul(
                                    out=ps[:, si, 0:SEGN], lhsT=lhsT,
                                    rhs=rhs, start=(k == 0),
                                    stop=(k == taps - 1))
                            # evacuate tile (skip junk col via 4-D AP) with
                            # fused per-channel sum
                            ps4 = bass.AP(
                                tensor=ps.tensor, offset=ps.offset,
                                ap=[ps.ap[0], [512, NSEG], [ROWB, SROWS],
                                    [1, W]])
                            y4 = y_all[:, ti, :].rearrange(
                                "p (s r w) -> p s r w", s=NSEG, r=SROWS)
                            if evac_eng == "s":
                                nc.scalar.activation(
                                    out=y4, in_=ps4, func=AF.Copy,
                                    accum_out=ssum[:, ti:ti + 1])
                            else:
                                eng(evac_eng).tensor_scalar(
                                    y4, ps4, 1.0, None, AL.mult,
                                    accum_out=ssum[:, ti:ti + 1])
                            # sum of squares
                            if ssq_eng == "s":
                                sc = opool.tile([128, HW], f16, tag="sc",
                                                name="sc")
                                nc.scalar.activation(
                                    out=sc[:], in_=y_all[:, ti, :],
                                    func=AF.Square,
                                    accum_out=ssq[:, ti:ti + 1])
                            else:
                                sc = opool.tile([128, HW], f16, tag="sc",
                                                name="sc")
                                eng(ssq_eng).scalar_tensor_tensor(
                                    out=sc[:], in0=y_all[:, ti, :], scalar=1.0,
                                    in1=y_all[:, ti, :], op0=AL.mult,
                                    op1=AL.mult, accum_out=ssq[:, ti:ti + 1])

                # ---- phase 2: reduce partials, sync-BN all-reduce ----
                NQ = BS * T
                for ch in range(NCHUNK):
                    nc.vector.tensor_reduce(
                        out=loc[:, ch:ch + 1],
                        in_=ssum[:, ch * NQ:(ch + 1) * NQ], axis=AX.X,
                        op=AL.add)
                    nc.vector.tensor_reduce(
                        out=loc[:, 3 + ch:4 + ch],
                        in_=ssq[:, ch * NQ:(ch + 1) * NQ], axis=AX.X,
                        op=AL.add)

                if with_collective:
                    cin = dpool.tile([128, 6], f32, tag="cin", name="cin")
                    cout = dpool.tile([128, 6], f32, tag="cout", name="cout")
                    nc.gpsimd.dma_start(out=cin[:], in_=loc[:])
                    nc.gpsimd.collective_compute(
                        "AllReduce", AL.add,
                        replica_groups=[list(range(n_cores))],
                        ins=[cin.opt()], outs=[cout.opt()])
                    nc.gpsimd.dma_start(out=gsum[:], in_=cout[:])
                else:
                    nc.vector.tensor_copy(out=gsum[:], in_=loc[:])

                # ---- phase 3: BN parameters (tiny [128,3] ops) ----
                mu, m2 = prm[:, 0:3], prm[:, 3:6]
                sq, var = prm[:, 6:9], prm[:, 9:12]
                veps, sv = prm[:, 12:15], prm[:, 15:18]
                r0_, r0s = prm[:, 18:21], prm[:, 21:24]
                xr, tcr = prm[:, 24:27], prm[:, 27:30]
                r1_, aa = prm[:, 30:33], prm[:, 33:36]
                t2, bb = prm[:, 36:39], prm[:, 39:42]
                inv_n = 1.0 / float(NTOT)
                nc.vector.tensor_scalar(mu, gsum[:, 0:3], inv_n, None,
                                        AL.mult)
                nc.vector.tensor_scalar(m2, gsum[:, 3:6], inv_n, None,
                                        AL.mult)
                nc.vector.tensor_tensor(out=sq, in0=mu, in1=mu, op=AL.mult)
                nc.vector.tensor_tensor(out=var, in0=m2, in1=sq,
                                        op=AL.subtract)
                nc.vector.tensor_scalar(veps, var, BN_EPS, None, AL.add)
                nc.scalar.activation(out=sv, in_=veps, func=AF.Sqrt)
                nc.vector.reciprocal(out=r0_, in_=sv)
                # one Newton step: r1 = r0*(1.5 - 0.5*x*r0^2)
                nc.vector.tensor_tensor(out=r0s, in0=r0_, in1=r0_, op=AL.mult)
                nc.vector.tensor_tensor(out=xr, in0=veps, in1=r0s, op=AL.mult)
                nc.vector.tensor_scalar(tcr, xr, -0.5, 1.5, AL.mult, AL.add)
                nc.vector.tensor_tensor(out=r1_, in0=r0_, in1=tcr, op=AL.mult)
                # a = r1*gamma ; b = beta - mu*a
                nc.vector.tensor_tensor(out=aa, in0=r1_, in1=gbsb[:, 0:3],
                                        op=AL.mult)
                nc.vector.tensor_tensor(out=t2, in0=mu, in1=aa, op=AL.mult)
                nc.vector.tensor_tensor(out=bb, in0=gbsb[:, 3:6], in1=t2,
                                        op=AL.subtract)

                # ---- phase 4: BN apply in place (engines cycled), then one
                # strided DMA per channel chunk ----
                n = 0
                for ch in range(NCHUNK):
                    for ti in range(ch * BS * T, (ch + 1) * BS * T):
                        e = bn_cycle[n % len(bn_cycle)]
                        if e == "s":
                            nc.scalar.activation(
                                out=y_all[:, ti, :], in_=y_all[:, ti, :],
                                func=AF.Identity,
                                bias=prm[:, 39 + ch:40 + ch],
                                scale=prm[:, 33 + ch:34 + ch])
                        else:
                            eng(e).tensor_scalar(
                                y_all[:, ti, :], y_all[:, ti, :],
                                prm[:, 33 + ch:34 + ch],
                                prm[:, 39 + ch:40 + ch],
                                AL.mult, AL.add)
                        n += 1
                    nc.sync.dma_start(
                        out=chunk_ap(y_d, ch),
                        in_=y_all[:, ch * BS * T:(ch + 1) * BS * T, :])
    nc.compile()
    return nc


def _host_prep(x, w, gamma, beta):
    """Shard/transform the full inputs into per-core in_maps."""
    x = np.asarray(x, dtype=np.float32).reshape(T, B, C, HW)
    w = np.asarray(w, dtype=np.float32)
    gamma = np.asarray(gamma, dtype=np.float32)
    beta = np.asarray(beta, dtype=np.float32)

    w9 = w.reshape(C, 9).astype(np.float16)
    idx = np.arange(128)
    wd = np.zeros((128, NCHUNK * 9, 128), dtype=np.float16)
    for ch in range(NCHUNK):
        for k in range(9):
            wd[idx, ch * 9 + k, idx] = w9[ch * 128:(ch + 1) * 128, k]

    gb = np.zeros((128, 6), dtype=np.float32)
    gb[:, 0:3] = gamma.reshape(NCHUNK, 128).T
    gb[:, 3:6] = beta.reshape(NCHUNK, 128).T

    in_maps = []
    for i in range(NCORES):
        xi = np.ascontiguousarray(x[:, i * BS:(i + 1) * BS])
        in_maps.append({"x": xi, "wd": wd, "gb": gb})
    return in_maps


def kernel(x, w, gamma, beta):
    from concourse.bass_utils import run_bass_kernel_spmd

    if "nc" not in _CACHE:
        _CACHE["nc"] = build_program()
    nc = _CACHE["nc"]

    in_maps = _host_prep(x, w, gamma, beta)
    res = run_bass_kernel_spmd(nc, in_maps, core_ids=list(range(NCORES)))

    out = np.empty((T, B, C, HW), dtype=np.float32)
    for i in range(NCORES):
        out[:, i * BS:(i + 1) * BS] = res.results[i]["y"]  # f16 -> f32 upcast
    return out.reshape(T, B, C, H, W)
